# revision 16
# baseline (speedup 1.0000x reference)
"""Trainium2 Bass kernel for nn_MultiHeadClassifier.

  logits[b, c] = sum_{(g,l): label_ids[g,l]==c} group_probs[b,g] *
                 (features[b] @ W[g,l] + b[g,l])

Data-parallel over batch (8 cores, 4096 rows each). Per core:
  * Host prep: transpose features/group_probs; sort the G*L=1024 head
    outputs by target class with NO padding (K_pad=1024, 8 chunks).
    Chunk class-bands may overlap by one boundary class; the scatter
    relies on per-element PSUM has_written semantics (accumulate where
    written, overwrite where pending) so overlapping bands need no
    extra matmuls.  Even/odd chunks use separate S matrices so a
    boundary class column never mixes rows of adjacent chunks.
  * Warmup: 12 junk matmuls at t=0 to lift the PE HAM clock gate
    (1.2 -> 2.4 GHz) before real work arrives.
  * Batch processed in phases of 512/1024 rows (512s at the ends to
    shorten pipeline fill and the final scatter tail); scatter for
    phase p-1 interleaves into phase p's chunk loop.
  * GEMM (PE, bf16): per (phase, chunk j) matmuls accumulate
    glT[128, w] over 4 K-chunks into a 2-bank PSUM pair tile.
  * M (PE): MT[gl, b] = E_j.T @ pT as a 0/1 matmul into a paired
    [128, 1024] PSUM tile.
  * ACT: one [128,1024] activation per (tp, j) drains PSUM with the
    per-partition bias; DVE: weighted = gb * MT (one [128,1024] mul).
  * Scatter (PE, bf16): per 128-row b-subtile and 512-col class half,
    band matmuls of wtj.T @ S accumulate into a single-bank psL tile.
  * Drains alternate ACT/DVE; output staged bf16 in [128, 4000] tiles
    and DMAd (8KB/partition descriptors) alternating the two HWDGE
    queues (sync/scalar) to a [128, 32000] DRAM layout that the host
    unpermutes.
"""
import os
import sys
import numpy as np
import ml_dtypes

for _p in ("/opt/trn_rl_repo",):
    if _p not in sys.path:
        sys.path.append(_p)

import concourse.bass as bass  # noqa: E402
import concourse.tile as tile  # noqa: E402
from concourse import bacc, mybir, bass_utils  # noqa: E402
from contextlib import ExitStack  # noqa: E402

F32 = mybir.dt.float32
BF16 = mybir.dt.bfloat16

B, F, G, L, C = 32768, 512, 16, 64, 1000
NCORE = 8
BC = B // NCORE          # 4096 batch rows per core
NT = BC // 512           # 8 b-tiles of 512
NP = NT // 2             # 4 t-pairs of 1024
KF = F // 128            # 4 feature chunks
NCH = (G * L) // 128     # 8 sorted-head chunks, no padding

LAST_EXEC_NS = None


def _host_prep(W, b, label_ids):
    lab = np.asarray(label_ids).reshape(-1).astype(np.int64)
    order = np.argsort(lab, kind="stable")
    slab = lab[order]

    bands = []
    lo = 0
    for j in range(NCH):
        last_c = int(slab[j * 128 + 127])
        hi = C if j == NCH - 1 else last_c + 1
        assert hi > lo
        bands.append((lo, hi))
        if j < NCH - 1:
            lo = last_c if int(slab[(j + 1) * 128]) == last_c else last_c + 1

    S = np.zeros((2, 128, C), dtype=ml_dtypes.bfloat16)
    for j in range(NCH):
        for r in range(128):
            S[j % 2, r, int(slab[j * 128 + r])] = 1.0

    Wflat = np.asarray(W).reshape(G * L, F)[order]
    WT = np.ascontiguousarray(Wflat.T.astype(ml_dtypes.bfloat16))
    bias = np.asarray(b).reshape(G * L)[order]
    biasT = np.zeros((128, NCH), dtype=np.float32)
    E = np.zeros((16, G * L), dtype=ml_dtypes.bfloat16)
    for p in range(G * L):
        biasT[p % 128, p // 128] = bias[p]
        E[order[p] // L, p] = 1.0
    return dict(bands=bands, S0=np.ascontiguousarray(S[0]),
                S1=np.ascontiguousarray(S[1]), WT=WT, biasT=biasT, E=E)


def _scatter_plans(bands):
    """Per class-half (bank) the ordered segment list (j, n0, n1, start, stop)."""
    halves = [[], []]
    for j, (lo, hi) in enumerate(bands):
        x = lo
        while x < hi:
            nxt = min(hi, (x // 512 + 1) * 512)
            halves[x // 512].append((j, x, nxt))
            x = nxt
    plans = []
    for h in (0, 1):
        segs = halves[h]
        plans.append([(j, n0, n1, i == 0, i == len(segs) - 1)
                      for i, (j, n0, n1) in enumerate(segs)])
    return plans


def _build_program(bands):
    plans = _scatter_plans(bands)
    nc = bacc.Bacc("TRN2", target_bir_lowering=False, debug=False,
                   num_devices=NCORE)
    xt_d = nc.dram_tensor("xt", [F, BC], BF16, kind="ExternalInput").ap()
    pt_d = nc.dram_tensor("pt", [16, BC], BF16, kind="ExternalInput").ap()
    wt_d = nc.dram_tensor("wt", [F, NCH * 128], BF16, kind="ExternalInput").ap()
    e_d = nc.dram_tensor("e", [16, NCH * 128], BF16, kind="ExternalInput").ap()
    bt_d = nc.dram_tensor("bt", [128, NCH], F32, kind="ExternalInput").ap()
    s0_d = nc.dram_tensor("s0", [128, C], BF16, kind="ExternalInput").ap()
    s1_d = nc.dram_tensor("s1", [128, C], BF16, kind="ExternalInput").ap()
    out_d = nc.dram_tensor("logits", [128, NP * 2 * 4000], BF16,
                           kind="ExternalOutput").ap()

    with tile.TileContext(nc) as tc, ExitStack() as ctx:
        const = ctx.enter_context(tc.tile_pool(name="const", bufs=1))
        psG = ctx.enter_context(tc.tile_pool(name="psG", bufs=2, space="PSUM"))
        psM = ctx.enter_context(tc.tile_pool(name="psM", bufs=1, space="PSUM"))
        psL = ctx.enter_context(tc.tile_pool(name="psL", bufs=2, space="PSUM"))
        sbG = ctx.enter_context(tc.tile_pool(name="sbG", bufs=4))
        sbW = ctx.enter_context(tc.tile_pool(name="sbW", bufs=18))
        sbO = ctx.enter_context(tc.tile_pool(name="sbO", bufs=3))

        junk = const.tile([128, 512], BF16, name="junk", tag="junk")
        nc.vector.memset(junk[:], 0.0)

        es = const.tile([16, NCH * 128], BF16, name="es", tag="es")
        nc.gpsimd.dma_start(es[:], e_d[:])
        pts = const.tile([16, BC], BF16, name="pts", tag="pts")
        nc.gpsimd.dma_start(pts[:], pt_d[:])
        # x in [128,1024] quarters, interleaved across sync+gpsimd queues
        # so each phase's slice lands well before its GEMMs need it
        wts = []
        xq = [[None] * 4 for _ in range(KF)]
        for q in range(4):
            for k in range(KF):
                t_ = const.tile([128, 1024], BF16, name=f"xq{k}_{q}",
                                tag=f"xq{k}_{q}")
                eng = nc.sync if k < 2 else nc.gpsimd
                eng.dma_start(t_[:], xt_d[k * 128:(k + 1) * 128,
                                          q * 1024:(q + 1) * 1024])
                xq[k][q] = t_
        for k in range(KF):
            t_ = const.tile([128, NCH * 128], BF16, name=f"wts{k}",
                            tag=f"wts{k}")
            nc.scalar.dma_start(t_[:], wt_d[k * 128:(k + 1) * 128, :])
            wts.append(t_)
        bts = const.tile([128, NCH], F32, name="bts", tag="bts")
        nc.scalar.dma_start(bts[:], bt_d[:])
        ss = []
        for i, sd in enumerate((s0_d, s1_d)):
            t_ = const.tile([128, C], BF16, name=f"ss{i}", tag=f"ss{i}")
            nc.scalar.dma_start(t_[:], sd[:])
            ss.append(t_)
        # HAM warmup: junk matmuls into a psG-pool tile (same tag -> no
        # extra PSUM footprint; the bank is recycled by the real GEMMs)
        warm = psG.tile([128, 1024], F32, name="psg", tag="psg")
        for _ in range(12):
            nc.tensor.matmul(warm[:, 0:512], junk[:, 0:128], junk[:],
                             start=True, stop=True)

        PHASES = [(0, 512), (512, 512), (1024, 1024), (2048, 1024),
                  (3072, 512), (3584, 512)]

        def x_slice(k, off, w):
            q, co = off // 1024, off % 1024
            return xq[k][q][:, co:co + w]

        wtjs = {}
        obs = {}
        drain_ct = [0]

        def scatter_unit(ph, u):
            off, w_ph = PHASES[ph]
            bs, h = u // 2, u % 2
            sb = off // 128 + bs            # global 128-row subtile index
            grp = sb // 4                   # output DMA group of 4 subtiles
            if sb % 4 == 0 and h == 0:
                obs[grp] = sbO.tile([128, 4000], BF16, name="ob", tag="ob")
            ob = obs[grp]
            w = 512 if h == 0 else C - 512
            pl = psL.tile([128, 512], F32, name="pl", tag="pl")
            for (j, n0, n1, st, sp) in plans[h]:
                nc.tensor.matmul(pl[:, n0 - h * 512:n1 - h * 512],
                                 wtjs[(ph, j)][:, bs * 128:(bs + 1) * 128],
                                 ss[j % 2][:, n0:n1], start=st, stop=sp,
                                 skip_group_check=True)
            dst = ob[:, (sb % 4) * 1000 + h * 512:
                     (sb % 4) * 1000 + h * 512 + w]
            if drain_ct[0] % 2 == 0:
                nc.scalar.activation(dst, pl[:, 0:w],
                                     mybir.ActivationFunctionType.Identity,
                                     bias=0.0, scale=1.0)
            else:
                nc.vector.tensor_copy(dst, pl[:, 0:w])
            drain_ct[0] += 1
            if grp == 7 and h == 1:
                # tail: don't hold the last group for one big DMA
                eng = nc.sync if sb % 2 == 0 else nc.scalar
                eng.dma_start(out_d[:, sb * 1000:(sb + 1) * 1000],
                              ob[:, (sb % 4) * 1000:(sb % 4) * 1000 + 1000])
                if sb % 4 == 3:
                    del obs[grp]
            elif sb % 4 == 3 and h == 1:
                eng = nc.sync if grp % 2 == 0 else nc.scalar
                eng.dma_start(out_d[:, grp * 4000:(grp + 1) * 4000], ob[:])
                del obs[grp]

        for ph, (off, w_ph) in enumerate(PHASES):
            prev_units = ((w_ph and ph > 0) and
                          [(ph - 1, u) for u in range(PHASES[ph - 1][1] // 64)]
                          or [])
            per_j = (len(prev_units) + NCH - 1) // NCH if prev_units else 0
            ui = 0
            for j in range(NCH):
                jsl = bass.ts(j, 128)
                psg = psG.tile([128, 1024], F32, name="psg", tag="psg")
                for k in range(KF):
                    for h in range(w_ph // 512):
                        nc.tensor.matmul(
                            psg[:, h * 512:(h + 1) * 512],
                            wts[k][:, jsl], x_slice(k, off + h * 512, 512),
                            start=(k == 0), stop=(k == KF - 1))
                pm = psM.tile([128, 1024], F32, name="pm", tag="pm")
                for h in range(w_ph // 512):
                    nc.tensor.matmul(pm[:, h * 512:(h + 1) * 512],
                                     es[:, jsl],
                                     pts[:, off + h * 512:off + h * 512 + 512],
                                     start=True, stop=True)
                gb = sbG.tile([128, 1024], BF16, name="gb", tag="gb")
                nc.scalar.activation(gb[:, 0:w_ph], psg[:, 0:w_ph],
                                     mybir.ActivationFunctionType.Identity,
                                     bias=bts[:, j:j + 1], scale=1.0)
                wtj = sbW.tile([128, 1024], BF16, name="wtj", tag="wtj")
                nc.vector.tensor_mul(wtj[:, 0:w_ph], gb[:, 0:w_ph],
                                     pm[:, 0:w_ph])
                wtjs[(ph, j)] = wtj
                for _ in range(per_j):
                    if ui < len(prev_units):
                        scatter_unit(*prev_units[ui])
                        ui += 1
            while ui < len(prev_units):
                scatter_unit(*prev_units[ui])
                ui += 1
            if ph > 0:
                for jj in range(NCH):
                    del wtjs[(ph - 1, jj)]
        last = len(PHASES) - 1
        for u in range(PHASES[last][1] // 64):
            scatter_unit(last, u)
    nc.finalize()
    return nc


def kernel(features, group_probs, W, b, label_ids):
    global LAST_EXEC_NS
    features = np.asarray(features, dtype=np.float32)
    group_probs = np.asarray(group_probs, dtype=np.float32)
    prep = _host_prep(W, b, label_ids)
    nc = _build_program(prep["bands"])

    XT = np.ascontiguousarray(features.T.astype(ml_dtypes.bfloat16))
    PT = np.ascontiguousarray(group_probs.T.astype(ml_dtypes.bfloat16))
    in_maps = []
    for c in range(NCORE):
        in_maps.append({
            "xt": np.ascontiguousarray(XT[:, c * BC:(c + 1) * BC]),
            "pt": np.ascontiguousarray(PT[:, c * BC:(c + 1) * BC]),
            "wt": prep["WT"],
            "e": prep["E"],
            "bt": prep["biasT"],
            "s0": prep["S0"],
            "s1": prep["S1"],
        })

    trace = bool(os.environ.get("BASS_TRACE"))
    if trace:
        bass_utils.upload_artifacts = lambda d: "local://skipped"
    try:
        res = bass_utils.run_bass_kernel_spmd(nc, in_maps,
                                              core_ids=list(range(NCORE)))
    except Exception:
        # transient NRT device errors have been observed; one retry
        res = bass_utils.run_bass_kernel_spmd(nc, in_maps,
                                              core_ids=list(range(NCORE)))
    if trace:
        LAST_EXEC_NS = res.exec_time_ns
        if res.exec_time_ns is not None:
            print(f"HW exec time: {res.exec_time_ns} ns")
        if res.instructions_and_trace is not None:
            print(f"Trace path: {res.instructions_and_trace[1]}")
        if res.profile_json is not None:
            print(f"Profile json: {res.profile_json}")

    parts = []
    for c in range(NCORE):
        arr = np.asarray(res.results[c]["logits"]).astype(np.float32)
        arr = arr.reshape(128, NP, 2, 4, 1000)
        parts.append(arr.transpose(1, 2, 3, 0, 4).reshape(BC, C))
    return np.ascontiguousarray(np.concatenate(parts, axis=0))


# revision 17
# speedup vs baseline: 1.0523x; 1.0523x over previous
"""Trainium2 Bass kernel for nn_MultiHeadClassifier.

  logits[b, c] = sum_{(g,l): label_ids[g,l]==c} group_probs[b,g] *
                 (features[b] @ W[g,l] + b[g,l])

Data-parallel over batch (8 cores, 4096 rows each). Per core:
  * Host prep: transpose features/group_probs; sort the G*L=1024 head
    outputs by target class with NO padding (K_pad=1024, 8 chunks).
    Chunk class-bands may overlap by one boundary class; the scatter
    relies on per-element PSUM has_written semantics (accumulate where
    written, overwrite where pending) so overlapping bands need no
    extra matmuls.  Even/odd chunks use separate S matrices so a
    boundary class column never mixes rows of adjacent chunks.
  * Warmup: 12 junk matmuls at t=0 to lift the PE HAM clock gate
    (1.2 -> 2.4 GHz) before real work arrives.
  * Batch processed in phases of 512/1024 rows (512s at the ends to
    shorten pipeline fill and the final scatter tail); scatter for
    phase p-1 interleaves into phase p's chunk loop.
  * GEMM (PE, bf16): per (phase, chunk j) matmuls accumulate
    glT[128, w] over 4 K-chunks into a 2-bank PSUM pair tile.
  * M (PE): MT[gl, b] = E_j.T @ pT as a 0/1 matmul into a paired
    [128, 1024] PSUM tile.
  * ACT: one [128,1024] activation per (tp, j) drains PSUM with the
    per-partition bias; DVE: weighted = gb * MT (one [128,1024] mul).
  * Scatter (PE, bf16): per 128-row b-subtile and 512-col class half,
    band matmuls of wtj.T @ S accumulate into a single-bank psL tile.
  * Drains alternate ACT/DVE; output staged bf16 in [128, 4000] tiles
    and DMAd (8KB/partition descriptors) alternating the two HWDGE
    queues (sync/scalar) to a [128, 32000] DRAM layout that the host
    unpermutes.
"""
import os
import sys
import numpy as np
import ml_dtypes

for _p in ("/opt/trn_rl_repo",):
    if _p not in sys.path:
        sys.path.append(_p)

import concourse.bass as bass  # noqa: E402
import concourse.tile as tile  # noqa: E402
from concourse import bacc, mybir, bass_utils  # noqa: E402
from contextlib import ExitStack  # noqa: E402

F32 = mybir.dt.float32
BF16 = mybir.dt.bfloat16

B, F, G, L, C = 32768, 512, 16, 64, 1000
NCORE = 8
BC = B // NCORE          # 4096 batch rows per core
NT = BC // 512           # 8 b-tiles of 512
NP = NT // 2             # 4 t-pairs of 1024
KF = F // 128            # 4 feature chunks
NCH = (G * L) // 128     # 8 sorted-head chunks, no padding

LAST_EXEC_NS = None


def _host_prep(W, b, label_ids):
    lab = np.asarray(label_ids).reshape(-1).astype(np.int64)
    order = np.argsort(lab, kind="stable")
    slab = lab[order]

    bands = []
    lo = 0
    for j in range(NCH):
        last_c = int(slab[j * 128 + 127])
        hi = C if j == NCH - 1 else last_c + 1
        assert hi > lo
        bands.append((lo, hi))
        if j < NCH - 1:
            lo = last_c if int(slab[(j + 1) * 128]) == last_c else last_c + 1

    S = np.zeros((2, 128, C), dtype=ml_dtypes.bfloat16)
    for j in range(NCH):
        for r in range(128):
            S[j % 2, r, int(slab[j * 128 + r])] = 1.0

    Wflat = np.asarray(W).reshape(G * L, F)[order]
    WT = np.ascontiguousarray(Wflat.T.astype(ml_dtypes.bfloat16))
    bias = np.asarray(b).reshape(G * L)[order]
    biasT = np.zeros((128, NCH), dtype=np.float32)
    E = np.zeros((16, G * L), dtype=ml_dtypes.bfloat16)
    for p in range(G * L):
        biasT[p % 128, p // 128] = bias[p]
        E[order[p] // L, p] = 1.0
    return dict(bands=bands, S0=np.ascontiguousarray(S[0]),
                S1=np.ascontiguousarray(S[1]), WT=WT, biasT=biasT, E=E)


def _scatter_plans(bands):
    """Per class-half (bank) the ordered segment list (j, n0, n1, start, stop)."""
    halves = [[], []]
    for j, (lo, hi) in enumerate(bands):
        x = lo
        while x < hi:
            nxt = min(hi, (x // 512 + 1) * 512)
            halves[x // 512].append((j, x, nxt))
            x = nxt
    plans = []
    for h in (0, 1):
        segs = halves[h]
        plans.append([(j, n0, n1, i == 0, i == len(segs) - 1)
                      for i, (j, n0, n1) in enumerate(segs)])
    return plans


def _build_program(bands):
    plans = _scatter_plans(bands)
    nc = bacc.Bacc("TRN2", target_bir_lowering=False, debug=False,
                   num_devices=NCORE)
    xt_d = nc.dram_tensor("xt", [F, BC], BF16, kind="ExternalInput").ap()
    pt_d = nc.dram_tensor("pt", [16, BC], BF16, kind="ExternalInput").ap()
    wt_d = nc.dram_tensor("wt", [F, NCH * 128], BF16, kind="ExternalInput").ap()
    e_d = nc.dram_tensor("e", [16, NCH * 128], BF16, kind="ExternalInput").ap()
    bt_d = nc.dram_tensor("bt", [128, NCH], F32, kind="ExternalInput").ap()
    s0_d = nc.dram_tensor("s0", [128, C], BF16, kind="ExternalInput").ap()
    s1_d = nc.dram_tensor("s1", [128, C], BF16, kind="ExternalInput").ap()
    out_d = nc.dram_tensor("logits", [128, NP * 2 * 4000], BF16,
                           kind="ExternalOutput").ap()

    with tile.TileContext(nc) as tc, ExitStack() as ctx:
        const = ctx.enter_context(tc.tile_pool(name="const", bufs=1))
        psG = ctx.enter_context(tc.tile_pool(name="psG", bufs=2, space="PSUM"))
        psM = ctx.enter_context(tc.tile_pool(name="psM", bufs=1, space="PSUM"))
        psL = ctx.enter_context(tc.tile_pool(name="psL", bufs=2, space="PSUM"))
        sbG = ctx.enter_context(tc.tile_pool(name="sbG", bufs=4))
        sbW = ctx.enter_context(tc.tile_pool(name="sbW", bufs=18))
        sbO = ctx.enter_context(tc.tile_pool(name="sbO", bufs=3))

        junk = const.tile([128, 512], BF16, name="junk", tag="junk")
        nc.vector.memset(junk[:], 0.0)

        es = const.tile([16, NCH * 128], BF16, name="es", tag="es")
        nc.gpsimd.dma_start(es[:], e_d[:])
        pts = const.tile([16, BC], BF16, name="pts", tag="pts")
        nc.gpsimd.dma_start(pts[:], pt_d[:])
        # xt on sync; weights/bias/S on scalar
        xta, xtb, wts = [], [], []
        for k in range(KF):
            t_ = const.tile([128, 1024], BF16, name=f"xta{k}", tag=f"xta{k}")
            nc.sync.dma_start(t_[:], xt_d[k * 128:(k + 1) * 128, 0:1024])
            xta.append(t_)
        for k in range(KF):
            t_ = const.tile([128, NCH * 128], BF16, name=f"wts{k}",
                            tag=f"wts{k}")
            nc.scalar.dma_start(t_[:], wt_d[k * 128:(k + 1) * 128, :])
            wts.append(t_)
        bts = const.tile([128, NCH], F32, name="bts", tag="bts")
        nc.scalar.dma_start(bts[:], bt_d[:])
        ss = []
        for i, sd in enumerate((s0_d, s1_d)):
            t_ = const.tile([128, C], BF16, name=f"ss{i}", tag=f"ss{i}")
            nc.scalar.dma_start(t_[:], sd[:])
            ss.append(t_)
        for k in range(KF):
            t_ = const.tile([128, 3072], BF16, name=f"xtb{k}", tag=f"xtb{k}")
            nc.sync.dma_start(t_[:], xt_d[k * 128:(k + 1) * 128, 1024:BC])
            xtb.append(t_)
        # HAM warmup: junk matmuls into a psG-pool tile (same tag -> no
        # extra PSUM footprint; the bank is recycled by the real GEMMs)
        warm = psG.tile([128, 1024], F32, name="psg", tag="psg")
        for _ in range(12):
            nc.tensor.matmul(warm[:, 0:512], junk[:, 0:128], junk[:],
                             start=True, stop=True)

        PHASES = [(0, 512), (512, 512), (1024, 1024), (2048, 1024),
                  (3072, 512), (3584, 512)]

        def x_slice(k, off, w):
            if off + w <= 1024:
                return xta[k][:, off:off + w]
            return xtb[k][:, off - 1024:off - 1024 + w]

        wtjs = {}
        obs = {}
        drain_ct = [0]

        def scatter_unit(ph, u):
            off, w_ph = PHASES[ph]
            bs, h = u // 2, u % 2
            sb = off // 128 + bs            # global 128-row subtile index
            grp = sb // 4                   # output DMA group of 4 subtiles
            if sb % 4 == 0 and h == 0:
                obs[grp] = sbO.tile([128, 4000], BF16, name="ob", tag="ob")
            ob = obs[grp]
            w = 512 if h == 0 else C - 512
            pl = psL.tile([128, 512], F32, name="pl", tag="pl")
            for (j, n0, n1, st, sp) in plans[h]:
                nc.tensor.matmul(pl[:, n0 - h * 512:n1 - h * 512],
                                 wtjs[(ph, j)][:, bs * 128:(bs + 1) * 128],
                                 ss[j % 2][:, n0:n1], start=st, stop=sp,
                                 skip_group_check=True)
            dst = ob[:, (sb % 4) * 1000 + h * 512:
                     (sb % 4) * 1000 + h * 512 + w]
            if drain_ct[0] % 2 == 0:
                nc.scalar.activation(dst, pl[:, 0:w],
                                     mybir.ActivationFunctionType.Identity,
                                     bias=0.0, scale=1.0)
            else:
                nc.vector.tensor_copy(dst, pl[:, 0:w])
            drain_ct[0] += 1
            if sb % 4 == 3 and h == 1:
                eng = nc.sync if grp % 2 == 0 else nc.scalar
                eng.dma_start(out_d[:, grp * 4000:(grp + 1) * 4000], ob[:])
                del obs[grp]

        for ph, (off, w_ph) in enumerate(PHASES):
            prev_units = ((w_ph and ph > 0) and
                          [(ph - 1, u) for u in range(PHASES[ph - 1][1] // 64)]
                          or [])
            per_j = (len(prev_units) + NCH - 1) // NCH if prev_units else 0
            ui = 0
            for j in range(NCH):
                jsl = bass.ts(j, 128)
                psg = psG.tile([128, 1024], F32, name="psg", tag="psg")
                for k in range(KF):
                    for h in range(w_ph // 512):
                        nc.tensor.matmul(
                            psg[:, h * 512:(h + 1) * 512],
                            wts[k][:, jsl], x_slice(k, off + h * 512, 512),
                            start=(k == 0), stop=(k == KF - 1))
                pm = psM.tile([128, 1024], F32, name="pm", tag="pm")
                for h in range(w_ph // 512):
                    nc.tensor.matmul(pm[:, h * 512:(h + 1) * 512],
                                     es[:, jsl],
                                     pts[:, off + h * 512:off + h * 512 + 512],
                                     start=True, stop=True)
                gb = sbG.tile([128, 1024], BF16, name="gb", tag="gb")
                nc.scalar.activation(gb[:, 0:w_ph], psg[:, 0:w_ph],
                                     mybir.ActivationFunctionType.Identity,
                                     bias=bts[:, j:j + 1], scale=1.0)
                wtj = sbW.tile([128, 1024], BF16, name="wtj", tag="wtj")
                nc.vector.tensor_mul(wtj[:, 0:w_ph], gb[:, 0:w_ph],
                                     pm[:, 0:w_ph])
                wtjs[(ph, j)] = wtj
                for _ in range(per_j):
                    if ui < len(prev_units):
                        scatter_unit(*prev_units[ui])
                        ui += 1
            while ui < len(prev_units):
                scatter_unit(*prev_units[ui])
                ui += 1
            if ph > 0:
                for jj in range(NCH):
                    del wtjs[(ph - 1, jj)]
        last = len(PHASES) - 1
        for u in range(PHASES[last][1] // 64):
            scatter_unit(last, u)
    nc.finalize()
    return nc


def kernel(features, group_probs, W, b, label_ids):
    global LAST_EXEC_NS
    features = np.asarray(features, dtype=np.float32)
    group_probs = np.asarray(group_probs, dtype=np.float32)
    prep = _host_prep(W, b, label_ids)
    nc = _build_program(prep["bands"])

    XT = np.ascontiguousarray(features.T.astype(ml_dtypes.bfloat16))
    PT = np.ascontiguousarray(group_probs.T.astype(ml_dtypes.bfloat16))
    in_maps = []
    for c in range(NCORE):
        in_maps.append({
            "xt": np.ascontiguousarray(XT[:, c * BC:(c + 1) * BC]),
            "pt": np.ascontiguousarray(PT[:, c * BC:(c + 1) * BC]),
            "wt": prep["WT"],
            "e": prep["E"],
            "bt": prep["biasT"],
            "s0": prep["S0"],
            "s1": prep["S1"],
        })

    trace = bool(os.environ.get("BASS_TRACE"))
    if trace:
        bass_utils.upload_artifacts = lambda d: "local://skipped"
    try:
        res = bass_utils.run_bass_kernel_spmd(nc, in_maps,
                                              core_ids=list(range(NCORE)))
    except Exception:
        # transient NRT device errors have been observed; one retry
        res = bass_utils.run_bass_kernel_spmd(nc, in_maps,
                                              core_ids=list(range(NCORE)))
    if trace:
        LAST_EXEC_NS = res.exec_time_ns
        if res.exec_time_ns is not None:
            print(f"HW exec time: {res.exec_time_ns} ns")
        if res.instructions_and_trace is not None:
            print(f"Trace path: {res.instructions_and_trace[1]}")
        if res.profile_json is not None:
            print(f"Profile json: {res.profile_json}")

    parts = []
    for c in range(NCORE):
        arr = np.asarray(res.results[c]["logits"]).astype(np.float32)
        arr = arr.reshape(128, NP, 2, 4, 1000)
        parts.append(arr.transpose(1, 2, 3, 0, 4).reshape(BC, C))
    return np.ascontiguousarray(np.concatenate(parts, axis=0))


# revision 18
# speedup vs baseline: 1.0535x; 1.0012x over previous
"""Trainium2 Bass kernel for nn_MultiHeadClassifier.

  logits[b, c] = sum_{(g,l): label_ids[g,l]==c} group_probs[b,g] *
                 (features[b] @ W[g,l] + b[g,l])

Data-parallel over batch (8 cores, 4096 rows each). Per core:
  * Host prep: transpose features/group_probs; sort the G*L=1024 head
    outputs by target class with NO padding (K_pad=1024, 8 chunks).
    Chunk class-bands may overlap by one boundary class; the scatter
    relies on per-element PSUM has_written semantics (accumulate where
    written, overwrite where pending) so overlapping bands need no
    extra matmuls.  Even/odd chunks use separate S matrices so a
    boundary class column never mixes rows of adjacent chunks.
  * Warmup: 12 junk matmuls at t=0 to lift the PE HAM clock gate
    (1.2 -> 2.4 GHz) before real work arrives.
  * Batch processed in phases of 512/1024 rows (512s at the ends to
    shorten pipeline fill and the final scatter tail); scatter for
    phase p-1 interleaves into phase p's chunk loop.
  * GEMM (PE, bf16): per (phase, chunk j) matmuls accumulate
    glT[128, w] over 4 K-chunks into a 2-bank PSUM pair tile.
  * M (PE): MT[gl, b] = E_j.T @ pT as a 0/1 matmul into a paired
    [128, 1024] PSUM tile.
  * ACT: one [128,1024] activation per (tp, j) drains PSUM with the
    per-partition bias; DVE: weighted = gb * MT (one [128,1024] mul).
  * Scatter (PE, bf16): per 128-row b-subtile and 512-col class half,
    band matmuls of wtj.T @ S accumulate into a single-bank psL tile.
  * Drains alternate ACT/DVE; output staged bf16 in [128, 4000] tiles
    and DMAd (8KB/partition descriptors) alternating the two HWDGE
    queues (sync/scalar) to a [128, 32000] DRAM layout that the host
    unpermutes.
"""
import os
import sys
import numpy as np
import ml_dtypes

for _p in ("/opt/trn_rl_repo",):
    if _p not in sys.path:
        sys.path.append(_p)

import concourse.bass as bass  # noqa: E402
import concourse.tile as tile  # noqa: E402
from concourse import bacc, mybir, bass_utils  # noqa: E402
from contextlib import ExitStack  # noqa: E402

F32 = mybir.dt.float32
BF16 = mybir.dt.bfloat16

B, F, G, L, C = 32768, 512, 16, 64, 1000
NCORE = 8
BC = B // NCORE          # 4096 batch rows per core
NT = BC // 512           # 8 b-tiles of 512
NP = NT // 2             # 4 t-pairs of 1024
KF = F // 128            # 4 feature chunks
NCH = (G * L) // 128     # 8 sorted-head chunks, no padding

LAST_EXEC_NS = None


def _host_prep(W, b, label_ids):
    lab = np.asarray(label_ids).reshape(-1).astype(np.int64)
    order = np.argsort(lab, kind="stable")
    slab = lab[order]

    bands = []
    lo = 0
    for j in range(NCH):
        last_c = int(slab[j * 128 + 127])
        hi = C if j == NCH - 1 else last_c + 1
        assert hi > lo
        bands.append((lo, hi))
        if j < NCH - 1:
            lo = last_c if int(slab[(j + 1) * 128]) == last_c else last_c + 1

    S = np.zeros((2, 128, C), dtype=ml_dtypes.bfloat16)
    for j in range(NCH):
        for r in range(128):
            S[j % 2, r, int(slab[j * 128 + r])] = 1.0

    Wflat = np.asarray(W).reshape(G * L, F)[order]
    WT = np.ascontiguousarray(Wflat.T.astype(ml_dtypes.bfloat16))
    bias = np.asarray(b).reshape(G * L)[order]
    biasT = np.zeros((128, NCH), dtype=np.float32)
    E = np.zeros((16, G * L), dtype=ml_dtypes.bfloat16)
    for p in range(G * L):
        biasT[p % 128, p // 128] = bias[p]
        E[order[p] // L, p] = 1.0
    return dict(bands=bands, S0=np.ascontiguousarray(S[0]),
                S1=np.ascontiguousarray(S[1]), WT=WT, biasT=biasT, E=E)


def _scatter_plans(bands):
    """Per class-half (bank) the ordered segment list (j, n0, n1, start, stop)."""
    halves = [[], []]
    for j, (lo, hi) in enumerate(bands):
        x = lo
        while x < hi:
            nxt = min(hi, (x // 512 + 1) * 512)
            halves[x // 512].append((j, x, nxt))
            x = nxt
    plans = []
    for h in (0, 1):
        segs = halves[h]
        plans.append([(j, n0, n1, i == 0, i == len(segs) - 1)
                      for i, (j, n0, n1) in enumerate(segs)])
    return plans


def _build_program(bands):
    plans = _scatter_plans(bands)
    nc = bacc.Bacc("TRN2", target_bir_lowering=False, debug=False,
                   num_devices=NCORE)
    xt_d = nc.dram_tensor("xt", [F, BC], BF16, kind="ExternalInput").ap()
    pt_d = nc.dram_tensor("pt", [16, BC], BF16, kind="ExternalInput").ap()
    wt_d = nc.dram_tensor("wt", [F, NCH * 128], BF16, kind="ExternalInput").ap()
    e_d = nc.dram_tensor("e", [16, NCH * 128], BF16, kind="ExternalInput").ap()
    bt_d = nc.dram_tensor("bt", [128, NCH], F32, kind="ExternalInput").ap()
    s0_d = nc.dram_tensor("s0", [128, C], BF16, kind="ExternalInput").ap()
    s1_d = nc.dram_tensor("s1", [128, C], BF16, kind="ExternalInput").ap()
    out_d = nc.dram_tensor("logits", [128, NP * 2 * 4000], BF16,
                           kind="ExternalOutput").ap()

    with tile.TileContext(nc) as tc, ExitStack() as ctx:
        const = ctx.enter_context(tc.tile_pool(name="const", bufs=1))
        psG = ctx.enter_context(tc.tile_pool(name="psG", bufs=2, space="PSUM"))
        psM = ctx.enter_context(tc.tile_pool(name="psM", bufs=1, space="PSUM"))
        psL = ctx.enter_context(tc.tile_pool(name="psL", bufs=2, space="PSUM"))
        sbG = ctx.enter_context(tc.tile_pool(name="sbG", bufs=4))
        sbW = ctx.enter_context(tc.tile_pool(name="sbW", bufs=18))
        sbO = ctx.enter_context(tc.tile_pool(name="sbO", bufs=3))

        junk = const.tile([128, 512], BF16, name="junk", tag="junk")
        nc.vector.memset(junk[:], 0.0)

        es = const.tile([16, NCH * 128], BF16, name="es", tag="es")
        nc.gpsimd.dma_start(es[:], e_d[:])
        pts = const.tile([16, BC], BF16, name="pts", tag="pts")
        nc.gpsimd.dma_start(pts[:], pt_d[:])
        # xt on sync; weights/bias/S on scalar
        xta, xtb, wts = [], [], []
        for k in range(KF):
            t_ = const.tile([128, 1024], BF16, name=f"xta{k}", tag=f"xta{k}")
            nc.sync.dma_start(t_[:], xt_d[k * 128:(k + 1) * 128, 0:1024])
            xta.append(t_)
        for k in range(KF):
            t_ = const.tile([128, NCH * 128], BF16, name=f"wts{k}",
                            tag=f"wts{k}")
            nc.scalar.dma_start(t_[:], wt_d[k * 128:(k + 1) * 128, :])
            wts.append(t_)
        bts = const.tile([128, NCH], F32, name="bts", tag="bts")
        nc.scalar.dma_start(bts[:], bt_d[:])
        ss = []
        for i, sd in enumerate((s0_d, s1_d)):
            t_ = const.tile([128, C], BF16, name=f"ss{i}", tag=f"ss{i}")
            nc.scalar.dma_start(t_[:], sd[:])
            ss.append(t_)
        for k in range(KF):
            t_ = const.tile([128, 3072], BF16, name=f"xtb{k}", tag=f"xtb{k}")
            nc.sync.dma_start(t_[:], xt_d[k * 128:(k + 1) * 128, 1024:BC])
            xtb.append(t_)
        # HAM warmup: junk matmuls into a psG-pool tile (same tag -> no
        # extra PSUM footprint; the bank is recycled by the real GEMMs)
        warm = psG.tile([128, 1024], F32, name="psg", tag="psg")
        for _ in range(12):
            nc.tensor.matmul(warm[:, 0:512], junk[:, 0:128], junk[:],
                             start=True, stop=True)

        PHASES = [(0, 1024), (1024, 1024), (2048, 1024),
                  (3072, 512), (3584, 512)]

        def x_slice(k, off, w):
            if off + w <= 1024:
                return xta[k][:, off:off + w]
            return xtb[k][:, off - 1024:off - 1024 + w]

        wtjs = {}
        obs = {}
        drain_ct = [0]

        def scatter_unit(ph, u):
            off, w_ph = PHASES[ph]
            bs, h = u // 2, u % 2
            sb = off // 128 + bs            # global 128-row subtile index
            grp = sb // 4                   # output DMA group of 4 subtiles
            if sb % 4 == 0 and h == 0:
                obs[grp] = sbO.tile([128, 4000], BF16, name="ob", tag="ob")
            ob = obs[grp]
            w = 512 if h == 0 else C - 512
            pl = psL.tile([128, 512], F32, name="pl", tag="pl")
            for (j, n0, n1, st, sp) in plans[h]:
                nc.tensor.matmul(pl[:, n0 - h * 512:n1 - h * 512],
                                 wtjs[(ph, j)][:, bs * 128:(bs + 1) * 128],
                                 ss[j % 2][:, n0:n1], start=st, stop=sp,
                                 skip_group_check=True)
            dst = ob[:, (sb % 4) * 1000 + h * 512:
                     (sb % 4) * 1000 + h * 512 + w]
            if drain_ct[0] % 2 == 0:
                nc.scalar.activation(dst, pl[:, 0:w],
                                     mybir.ActivationFunctionType.Identity,
                                     bias=0.0, scale=1.0)
            else:
                nc.vector.tensor_copy(dst, pl[:, 0:w])
            drain_ct[0] += 1
            if sb % 4 == 3 and h == 1:
                eng = nc.sync if grp % 2 == 0 else nc.scalar
                eng.dma_start(out_d[:, grp * 4000:(grp + 1) * 4000], ob[:])
                del obs[grp]

        for ph, (off, w_ph) in enumerate(PHASES):
            prev_units = ((w_ph and ph > 0) and
                          [(ph - 1, u) for u in range(PHASES[ph - 1][1] // 64)]
                          or [])
            per_j = (len(prev_units) + NCH - 1) // NCH if prev_units else 0
            ui = 0
            for j in range(NCH):
                jsl = bass.ts(j, 128)
                psg = psG.tile([128, 1024], F32, name="psg", tag="psg")
                for k in range(KF):
                    for h in range(w_ph // 512):
                        nc.tensor.matmul(
                            psg[:, h * 512:(h + 1) * 512],
                            wts[k][:, jsl], x_slice(k, off + h * 512, 512),
                            start=(k == 0), stop=(k == KF - 1))
                pm = psM.tile([128, 1024], F32, name="pm", tag="pm")
                for h in range(w_ph // 512):
                    nc.tensor.matmul(pm[:, h * 512:(h + 1) * 512],
                                     es[:, jsl],
                                     pts[:, off + h * 512:off + h * 512 + 512],
                                     start=True, stop=True)
                gb = sbG.tile([128, 1024], BF16, name="gb", tag="gb")
                nc.scalar.activation(gb[:, 0:w_ph], psg[:, 0:w_ph],
                                     mybir.ActivationFunctionType.Identity,
                                     bias=bts[:, j:j + 1], scale=1.0)
                wtj = sbW.tile([128, 1024], BF16, name="wtj", tag="wtj")
                nc.vector.tensor_mul(wtj[:, 0:w_ph], gb[:, 0:w_ph],
                                     pm[:, 0:w_ph])
                wtjs[(ph, j)] = wtj
                for _ in range(per_j):
                    if ui < len(prev_units):
                        scatter_unit(*prev_units[ui])
                        ui += 1
            while ui < len(prev_units):
                scatter_unit(*prev_units[ui])
                ui += 1
            if ph > 0:
                for jj in range(NCH):
                    del wtjs[(ph - 1, jj)]
        last = len(PHASES) - 1
        for u in range(PHASES[last][1] // 64):
            scatter_unit(last, u)
    nc.finalize()
    return nc


def kernel(features, group_probs, W, b, label_ids):
    global LAST_EXEC_NS
    features = np.asarray(features, dtype=np.float32)
    group_probs = np.asarray(group_probs, dtype=np.float32)
    prep = _host_prep(W, b, label_ids)
    nc = _build_program(prep["bands"])

    XT = np.ascontiguousarray(features.T.astype(ml_dtypes.bfloat16))
    PT = np.ascontiguousarray(group_probs.T.astype(ml_dtypes.bfloat16))
    in_maps = []
    for c in range(NCORE):
        in_maps.append({
            "xt": np.ascontiguousarray(XT[:, c * BC:(c + 1) * BC]),
            "pt": np.ascontiguousarray(PT[:, c * BC:(c + 1) * BC]),
            "wt": prep["WT"],
            "e": prep["E"],
            "bt": prep["biasT"],
            "s0": prep["S0"],
            "s1": prep["S1"],
        })

    trace = bool(os.environ.get("BASS_TRACE"))
    if trace:
        bass_utils.upload_artifacts = lambda d: "local://skipped"
    try:
        res = bass_utils.run_bass_kernel_spmd(nc, in_maps,
                                              core_ids=list(range(NCORE)))
    except Exception:
        # transient NRT device errors have been observed; one retry
        res = bass_utils.run_bass_kernel_spmd(nc, in_maps,
                                              core_ids=list(range(NCORE)))
    if trace:
        LAST_EXEC_NS = res.exec_time_ns
        if res.exec_time_ns is not None:
            print(f"HW exec time: {res.exec_time_ns} ns")
        if res.instructions_and_trace is not None:
            print(f"Trace path: {res.instructions_and_trace[1]}")
        if res.profile_json is not None:
            print(f"Profile json: {res.profile_json}")

    parts = []
    for c in range(NCORE):
        arr = np.asarray(res.results[c]["logits"]).astype(np.float32)
        arr = arr.reshape(128, NP, 2, 4, 1000)
        parts.append(arr.transpose(1, 2, 3, 0, 4).reshape(BC, C))
    return np.ascontiguousarray(np.concatenate(parts, axis=0))


# revision 19
# speedup vs baseline: 1.0553x; 1.0017x over previous
"""Trainium2 Bass kernel for nn_MultiHeadClassifier.

  logits[b, c] = sum_{(g,l): label_ids[g,l]==c} group_probs[b,g] *
                 (features[b] @ W[g,l] + b[g,l])

Data-parallel over batch (8 cores, 4096 rows each). Per core:
  * Host prep: transpose features/group_probs; sort the G*L=1024 head
    outputs by target class with NO padding (K_pad=1024, 8 chunks).
    Chunk class-bands may overlap by one boundary class; the scatter
    relies on per-element PSUM has_written semantics (accumulate where
    written, overwrite where pending) so overlapping bands need no
    extra matmuls.  Even/odd chunks use separate S matrices so a
    boundary class column never mixes rows of adjacent chunks.
  * Warmup: 12 junk matmuls at t=0 to lift the PE HAM clock gate
    (1.2 -> 2.4 GHz) before real work arrives.
  * Batch processed in phases of 512/1024 rows (512s at the ends to
    shorten pipeline fill and the final scatter tail); scatter for
    phase p-1 interleaves into phase p's chunk loop.
  * GEMM (PE, bf16): per (phase, chunk j) matmuls accumulate
    glT[128, w] over 4 K-chunks into a 2-bank PSUM pair tile.
  * M (PE): MT[gl, b] = E_j.T @ pT as a 0/1 matmul into a paired
    [128, 1024] PSUM tile.
  * ACT: one [128,1024] activation per (tp, j) drains PSUM with the
    per-partition bias; DVE: weighted = gb * MT (one [128,1024] mul).
  * Scatter (PE, bf16): per 128-row b-subtile and 512-col class half,
    band matmuls of wtj.T @ S accumulate into a single-bank psL tile.
  * Drains alternate ACT/DVE; output staged bf16 in [128, 4000] tiles
    and DMAd (8KB/partition descriptors) alternating the two HWDGE
    queues (sync/scalar) to a [128, 32000] DRAM layout that the host
    unpermutes.
"""
import os
import sys
import numpy as np
import ml_dtypes

for _p in ("/opt/trn_rl_repo",):
    if _p not in sys.path:
        sys.path.append(_p)

import concourse.bass as bass  # noqa: E402
import concourse.tile as tile  # noqa: E402
from concourse import bacc, mybir, bass_utils  # noqa: E402
from contextlib import ExitStack  # noqa: E402

F32 = mybir.dt.float32
BF16 = mybir.dt.bfloat16

B, F, G, L, C = 32768, 512, 16, 64, 1000
NCORE = 8
BC = B // NCORE          # 4096 batch rows per core
NT = BC // 512           # 8 b-tiles of 512
NP = NT // 2             # 4 t-pairs of 1024
KF = F // 128            # 4 feature chunks
NCH = (G * L) // 128     # 8 sorted-head chunks, no padding

LAST_EXEC_NS = None


def _host_prep(W, b, label_ids):
    lab = np.asarray(label_ids).reshape(-1).astype(np.int64)
    order = np.argsort(lab, kind="stable")
    slab = lab[order]

    bands = []
    lo = 0
    for j in range(NCH):
        last_c = int(slab[j * 128 + 127])
        hi = C if j == NCH - 1 else last_c + 1
        assert hi > lo
        bands.append((lo, hi))
        if j < NCH - 1:
            lo = last_c if int(slab[(j + 1) * 128]) == last_c else last_c + 1

    S = np.zeros((2, 128, C), dtype=ml_dtypes.bfloat16)
    for j in range(NCH):
        for r in range(128):
            S[j % 2, r, int(slab[j * 128 + r])] = 1.0

    Wflat = np.asarray(W).reshape(G * L, F)[order]
    WT = np.ascontiguousarray(Wflat.T.astype(ml_dtypes.bfloat16))
    bias = np.asarray(b).reshape(G * L)[order]
    biasT = np.zeros((128, NCH), dtype=np.float32)
    E = np.zeros((16, G * L), dtype=ml_dtypes.bfloat16)
    for p in range(G * L):
        biasT[p % 128, p // 128] = bias[p]
        E[order[p] // L, p] = 1.0
    return dict(bands=bands, S0=np.ascontiguousarray(S[0]),
                S1=np.ascontiguousarray(S[1]), WT=WT, biasT=biasT, E=E)


def _scatter_plans(bands):
    """Per class-half (bank) the ordered segment list (j, n0, n1, start, stop)."""
    halves = [[], []]
    for j, (lo, hi) in enumerate(bands):
        x = lo
        while x < hi:
            nxt = min(hi, (x // 512 + 1) * 512)
            halves[x // 512].append((j, x, nxt))
            x = nxt
    plans = []
    for h in (0, 1):
        segs = halves[h]
        plans.append([(j, n0, n1, i == 0, i == len(segs) - 1)
                      for i, (j, n0, n1) in enumerate(segs)])
    return plans


def _build_program(bands):
    plans = _scatter_plans(bands)
    nc = bacc.Bacc("TRN2", target_bir_lowering=False, debug=False,
                   num_devices=NCORE)
    xt_d = nc.dram_tensor("xt", [F, BC], BF16, kind="ExternalInput").ap()
    pt_d = nc.dram_tensor("pt", [16, BC], BF16, kind="ExternalInput").ap()
    wt_d = nc.dram_tensor("wt", [F, NCH * 128], BF16, kind="ExternalInput").ap()
    e_d = nc.dram_tensor("e", [16, NCH * 128], BF16, kind="ExternalInput").ap()
    bt_d = nc.dram_tensor("bt", [128, NCH], F32, kind="ExternalInput").ap()
    s0_d = nc.dram_tensor("s0", [128, C], BF16, kind="ExternalInput").ap()
    s1_d = nc.dram_tensor("s1", [128, C], BF16, kind="ExternalInput").ap()
    out_d = nc.dram_tensor("logits", [128, NP * 2 * 4000], BF16,
                           kind="ExternalOutput").ap()

    with tile.TileContext(nc) as tc, ExitStack() as ctx:
        const = ctx.enter_context(tc.tile_pool(name="const", bufs=1))
        psG = ctx.enter_context(tc.tile_pool(name="psG", bufs=2, space="PSUM"))
        psM = ctx.enter_context(tc.tile_pool(name="psM", bufs=1, space="PSUM"))
        psL = ctx.enter_context(tc.tile_pool(name="psL", bufs=2, space="PSUM"))
        sbG = ctx.enter_context(tc.tile_pool(name="sbG", bufs=4))
        sbW = ctx.enter_context(tc.tile_pool(name="sbW", bufs=18))
        sbO = ctx.enter_context(tc.tile_pool(name="sbO", bufs=3))

        junk = const.tile([128, 512], BF16, name="junk", tag="junk")
        nc.vector.memset(junk[:], 0.0)

        es = const.tile([16, NCH * 128], BF16, name="es", tag="es")
        nc.gpsimd.dma_start(es[:], e_d[:])
        pts = const.tile([16, BC], BF16, name="pts", tag="pts")
        nc.gpsimd.dma_start(pts[:], pt_d[:])
        # xt on sync; weights/bias/S on scalar
        xta, xtb, wts = [], [], []
        for k in range(KF):
            t_ = const.tile([128, 1024], BF16, name=f"xta{k}", tag=f"xta{k}")
            nc.sync.dma_start(t_[:], xt_d[k * 128:(k + 1) * 128, 0:1024])
            xta.append(t_)
        for k in range(KF):
            t_ = const.tile([128, NCH * 128], BF16, name=f"wts{k}",
                            tag=f"wts{k}")
            nc.scalar.dma_start(t_[:], wt_d[k * 128:(k + 1) * 128, :])
            wts.append(t_)
        bts = const.tile([128, NCH], F32, name="bts", tag="bts")
        nc.scalar.dma_start(bts[:], bt_d[:])
        ss = []
        for i, sd in enumerate((s0_d, s1_d)):
            t_ = const.tile([128, C], BF16, name=f"ss{i}", tag=f"ss{i}")
            nc.scalar.dma_start(t_[:], sd[:])
            ss.append(t_)
        xbr = []
        for k in range(KF):
            t_ = const.tile([128, 1024], BF16, name=f"xtb{k}", tag=f"xtb{k}")
            nc.sync.dma_start(t_[:], xt_d[k * 128:(k + 1) * 128, 1024:2048])
            xtb.append(t_)
        for k in range(KF):
            t_ = const.tile([128, 2048], BF16, name=f"xbr{k}", tag=f"xbr{k}")
            nc.sync.dma_start(t_[:], xt_d[k * 128:(k + 1) * 128, 2048:BC])
            xbr.append(t_)
        # HAM warmup: junk matmuls into a psG-pool tile (same tag -> no
        # extra PSUM footprint; the bank is recycled by the real GEMMs)
        warm = psG.tile([128, 1024], F32, name="psg", tag="psg")
        for _ in range(12):
            nc.tensor.matmul(warm[:, 0:512], junk[:, 0:128], junk[:],
                             start=True, stop=True)

        PHASES = [(0, 1024), (1024, 1024), (2048, 1024),
                  (3072, 512), (3584, 512)]

        def x_slice(k, off, w):
            if off + w <= 1024:
                return xta[k][:, off:off + w]
            if off + w <= 2048:
                return xtb[k][:, off - 1024:off - 1024 + w]
            return xbr[k][:, off - 2048:off - 2048 + w]

        wtjs = {}
        obs = {}
        drain_ct = [0]

        def scatter_unit(ph, u):
            off, w_ph = PHASES[ph]
            bs, h = u // 2, u % 2
            sb = off // 128 + bs            # global 128-row subtile index
            grp = sb // 4                   # output DMA group of 4 subtiles
            if sb % 4 == 0 and h == 0:
                obs[grp] = sbO.tile([128, 4000], BF16, name="ob", tag="ob")
            ob = obs[grp]
            w = 512 if h == 0 else C - 512
            pl = psL.tile([128, 512], F32, name="pl", tag="pl")
            for (j, n0, n1, st, sp) in plans[h]:
                nc.tensor.matmul(pl[:, n0 - h * 512:n1 - h * 512],
                                 wtjs[(ph, j)][:, bs * 128:(bs + 1) * 128],
                                 ss[j % 2][:, n0:n1], start=st, stop=sp,
                                 skip_group_check=True)
            dst = ob[:, (sb % 4) * 1000 + h * 512:
                     (sb % 4) * 1000 + h * 512 + w]
            if drain_ct[0] % 2 == 0:
                nc.scalar.activation(dst, pl[:, 0:w],
                                     mybir.ActivationFunctionType.Identity,
                                     bias=0.0, scale=1.0)
            else:
                nc.vector.tensor_copy(dst, pl[:, 0:w])
            drain_ct[0] += 1
            if sb % 4 == 3 and h == 1:
                eng = nc.sync if grp % 2 == 0 else nc.scalar
                eng.dma_start(out_d[:, grp * 4000:(grp + 1) * 4000], ob[:])
                del obs[grp]

        for ph, (off, w_ph) in enumerate(PHASES):
            prev_units = ((w_ph and ph > 0) and
                          [(ph - 1, u) for u in range(PHASES[ph - 1][1] // 64)]
                          or [])
            per_j = (len(prev_units) + NCH - 1) // NCH if prev_units else 0
            ui = 0
            for j in range(NCH):
                jsl = bass.ts(j, 128)
                psg = psG.tile([128, 1024], F32, name="psg", tag="psg")
                for k in range(KF):
                    for h in range(w_ph // 512):
                        nc.tensor.matmul(
                            psg[:, h * 512:(h + 1) * 512],
                            wts[k][:, jsl], x_slice(k, off + h * 512, 512),
                            start=(k == 0), stop=(k == KF - 1))
                pm = psM.tile([128, 1024], F32, name="pm", tag="pm")
                for h in range(w_ph // 512):
                    nc.tensor.matmul(pm[:, h * 512:(h + 1) * 512],
                                     es[:, jsl],
                                     pts[:, off + h * 512:off + h * 512 + 512],
                                     start=True, stop=True)
                gb = sbG.tile([128, 1024], BF16, name="gb", tag="gb")
                nc.scalar.activation(gb[:, 0:w_ph], psg[:, 0:w_ph],
                                     mybir.ActivationFunctionType.Identity,
                                     bias=bts[:, j:j + 1], scale=1.0)
                wtj = sbW.tile([128, 1024], BF16, name="wtj", tag="wtj")
                nc.vector.tensor_mul(wtj[:, 0:w_ph], gb[:, 0:w_ph],
                                     pm[:, 0:w_ph])
                wtjs[(ph, j)] = wtj
                for _ in range(per_j):
                    if ui < len(prev_units):
                        scatter_unit(*prev_units[ui])
                        ui += 1
            while ui < len(prev_units):
                scatter_unit(*prev_units[ui])
                ui += 1
            if ph > 0:
                for jj in range(NCH):
                    del wtjs[(ph - 1, jj)]
        last = len(PHASES) - 1
        for u in range(PHASES[last][1] // 64):
            scatter_unit(last, u)
    nc.finalize()
    return nc


def kernel(features, group_probs, W, b, label_ids):
    global LAST_EXEC_NS
    features = np.asarray(features, dtype=np.float32)
    group_probs = np.asarray(group_probs, dtype=np.float32)
    prep = _host_prep(W, b, label_ids)
    nc = _build_program(prep["bands"])

    XT = np.ascontiguousarray(features.T.astype(ml_dtypes.bfloat16))
    PT = np.ascontiguousarray(group_probs.T.astype(ml_dtypes.bfloat16))
    in_maps = []
    for c in range(NCORE):
        in_maps.append({
            "xt": np.ascontiguousarray(XT[:, c * BC:(c + 1) * BC]),
            "pt": np.ascontiguousarray(PT[:, c * BC:(c + 1) * BC]),
            "wt": prep["WT"],
            "e": prep["E"],
            "bt": prep["biasT"],
            "s0": prep["S0"],
            "s1": prep["S1"],
        })

    trace = bool(os.environ.get("BASS_TRACE"))
    if trace:
        bass_utils.upload_artifacts = lambda d: "local://skipped"
    try:
        res = bass_utils.run_bass_kernel_spmd(nc, in_maps,
                                              core_ids=list(range(NCORE)))
    except Exception:
        # transient NRT device errors have been observed; one retry
        res = bass_utils.run_bass_kernel_spmd(nc, in_maps,
                                              core_ids=list(range(NCORE)))
    if trace:
        LAST_EXEC_NS = res.exec_time_ns
        if res.exec_time_ns is not None:
            print(f"HW exec time: {res.exec_time_ns} ns")
        if res.instructions_and_trace is not None:
            print(f"Trace path: {res.instructions_and_trace[1]}")
        if res.profile_json is not None:
            print(f"Profile json: {res.profile_json}")

    parts = []
    for c in range(NCORE):
        arr = np.asarray(res.results[c]["logits"]).astype(np.float32)
        arr = arr.reshape(128, NP, 2, 4, 1000)
        parts.append(arr.transpose(1, 2, 3, 0, 4).reshape(BC, C))
    return np.ascontiguousarray(np.concatenate(parts, axis=0))
